# revision 10
# baseline (speedup 1.0000x reference)
"""Causal multi-head attention block (qkv proj + causal softmax attention + out proj)
for Trainium2, sharded over 8 NeuronCores: data-parallel over batch (2) x
tensor-parallel over heads (4 heads per core of 16).

v2 layout: heads are PAIR-MAJOR in the partition dim. qT/kT tiles hold head 2p
in partitions 0:64 and head 2p+1 in partitions 64:128, so the scores matmuls
run as 64x128 row-tiled pairs (tile_position (0,0) / (64,0)) -- two heads
concurrently on the PE array.  Score tiles are [128, 1024] two-bank PSUM tiles
covering a kt-PAIR, halving the EXP instruction count on the ACT engine.  The
causal mask is a multiplicative 0/1 bf16 mask applied post-exp on DVE.  qkv,
attention, and out-proj emission is interleaved so the PE stream stays dense
while ACT digests the exps.

Each core computes, for its batch b and its 4 heads:
  qT,kT [hd-pair 128, S] and v [S, 4, 65]  (qkv projection; 65th col = ones)
  ST    [k, q] row-tiled scores, P = exp(ST) * tri-mask
  attnT [65, q] = [v | 1].T @ P   (row 64 = softmax denominator)
  an    = attnT / denom
  out_partial [S, D] = an.T @ owT  (row-parallel out proj)
Host sums the 4 per-core partials of each batch.
"""

import os
import sys

import numpy as np

sys.path.insert(0, "/opt/trn_rl_repo")

import concourse.bass as bass
import concourse.tile as tile
from concourse import bacc, mybir
from concourse.bass import MemorySpace
from concourse.bass_utils import run_bass_kernel_spmd

F32 = mybir.dt.float32
BF16 = mybir.dt.bfloat16
EXP = mybir.ActivationFunctionType.Exp

B, S, D = 2, 2048, 1024
H, HD = 16, 64
NCORES = 8
NH = 4          # heads per core
NP = 2          # head pairs per core
SCALE = HD ** -0.5

N_DT = D // 128          # 8 d-tiles of 128
N_ST = S // 128          # 16 seq tiles of 128
N_CH = S // 512          # 4 q chunks of 512
VW = NH * HD             # 256 v columns

KDT = os.environ.get("KDT", "bf16")
MM_DT = BF16


def _emit(tc, nc, xT_d, wT_d, owT_d, mask_d, out_d):
    import contextlib

    ctx = contextlib.ExitStack()
    with ctx:
        # ---------------- pools ----------------
        sb = ctx.enter_context(tc.tile_pool(name="sb", bufs=1))
        p_pool = ctx.enter_context(tc.tile_pool(name="psb", bufs=5))
        an_pool = ctx.enter_context(tc.tile_pool(name="attn_n", bufs=4))
        sm_pool = ctx.enter_context(tc.tile_pool(name="smalls", bufs=8))
        out_pool = ctx.enter_context(tc.tile_pool(name="outsb", bufs=3))
        # PSUM: big 2-bank tiles (qkv groups + score kt-pairs): 2x2 banks;
        # at: attnT accumulators [65,512]: 2x1; out: outproj [128,1024]: 1x2.
        ps_big = ctx.enter_context(
            tc.tile_pool(name="ps_big", bufs=2, space=MemorySpace.PSUM))
        ps_at = ctx.enter_context(
            tc.tile_pool(name="ps_at", bufs=2, space=MemorySpace.PSUM))
        ps_out = ctx.enter_context(
            tc.tile_pool(name="ps_out", bufs=1, space=MemorySpace.PSUM))

        # ---------------- static SBUF ----------------
        # qk_sb: f0=q_pair0, f1=q_pair1, f2=k_pair0, f3=k_pair1;
        # rows 0:64 = head 2p, rows 64:128 = head 2p+1.
        qk_sb = [sb.tile([128, S], MM_DT, tag=f"qk{i}", name=f"qk{i}")
                 for i in range(4)]
        # v: per seq-tile [128, 4 heads, 65] (64 v cols + ones col)
        v_sb = [sb.tile([128, NH, HD + 1], MM_DT, tag=f"v{i}", name=f"v{i}")
                for i in range(N_ST)]
        mask_sb = sb.tile([128, 128], MM_DT)
        owT_sb = [sb.tile([128, D], MM_DT, tag=f"ow{i}", name=f"ow{i}")
                  for i in range(NP)]
        xT_sb = [sb.tile([128, S], MM_DT, tag=f"x{i}", name=f"x{i}")
                 for i in range(N_DT)]
        wT_sb = [sb.tile([128, 3 * VW], MM_DT, tag=f"w{i}", name=f"w{i}")
                 for i in range(N_DT)]
        warm_sb = sb.tile([128, 512], MM_DT)

        # ---------------- input DMAs ----------------
        nc.sync.dma_start(out=mask_sb, in_=mask_d)
        for p in range(NP):
            nc.sync.dma_start(out=owT_sb[p], in_=owT_d[p * 128:(p + 1) * 128, :])
        for i in range(N_DT):
            nc.sync.dma_start(out=wT_sb[i], in_=wT_d[i * 128:(i + 1) * 128, :])
        for sp in range(2):
            csl = slice(sp * 1024, (sp + 1) * 1024)
            for i in range(N_DT):
                nc.sync.dma_start(out=xT_sb[i][:, csl],
                                  in_=xT_d[i * 128:(i + 1) * 128, csl])

        # HAM warm-up: dependency-free matmuls run while the input DMAs
        # stream so the PE clock-gate is 8/8 when real work starts.  A tiny
        # EXP preloads the ACT table set off the critical path.
        nc.vector.memset(warm_sb, 0.0)
        wu_ps = ps_at.tile([128, 512], F32, tag="at", name="wu_ps")
        for _ in range(18):
            nc.tensor.matmul(wu_ps, warm_sb[:, 0:128], warm_sb,
                             start=True, stop=True)
        warm_exp = sm_pool.tile([1, 8], F32, tag="wexp")
        nc.scalar.activation(warm_exp, wu_ps[0:1, 0:8], EXP)
        # v ones columns (written once; v copies never touch col 64)
        for st in range(N_ST):
            nc.gpsimd.memset(v_sb[st][:, :, HD:HD + 1], 1.0)

        # ---------------- qkv projection groups ----------------
        def emit_qk_group(f, sp):
            # qT/kT: out[f 128, s 1024] += wT[d, f].T @ xT[d, s]
            pss = ps_big.tile([128, 1024], F32, tag="big", name="psqk")
            for half in range(2):
                sch = 2 * sp + half
                for d in range(N_DT):
                    nc.tensor.matmul(
                        pss[:, half * 512:(half + 1) * 512],
                        wT_sb[d][:, f * 128:(f + 1) * 128],
                        xT_sb[d][:, sch * 512:(sch + 1) * 512],
                        start=(d == 0),
                        stop=(d == N_DT - 1),
                    )
            nc.vector.tensor_copy(
                qk_sb[f][:, sp * 1024:(sp + 1) * 1024], pss)

        def emit_v_group(sp, half):
            # v: out[s 128, 256] += xT[d, s].T @ wvT[d, 256]; two s-tiles
            # share one 2-bank tile.
            psv = ps_big.tile([128, 1024], F32, tag="big", name="psv")
            for q in range(2):
                st = 8 * sp + 2 * half + q
                for d in range(N_DT):
                    nc.tensor.matmul(
                        psv[:, q * 512:q * 512 + VW],
                        xT_sb[d][:, st * 128:(st + 1) * 128],
                        wT_sb[d][:, 2 * VW:3 * VW],
                        start=(d == 0),
                        stop=(d == N_DT - 1),
                    )
            for q in range(2):
                st = 8 * sp + 2 * half + q
                nc.scalar.copy(
                    v_sb[st][:, :, 0:HD],
                    psv[:, q * 512:q * 512 + VW].rearrange(
                        "p (h d) -> p h d", h=NH),
                )

        # ---------------- attention ----------------
        an_hist = {}

        def emit_attention(qc, fillers):
            """Scores(row-tiled pairs) + exp + mask + attnT for chunk qc.
            Pulls one filler closure per kt-pair step to keep PE dense."""
            n_kt = 4 * (qc + 1)
            G = n_kt // 2
            an = [an_pool.tile([128, 512], MM_DT, tag=f"an{p}", name=f"an{p}")
                  for p in range(NP)]
            an_hist[qc] = an
            for p in range(NP):
                at_ps = [ps_at.tile([128, 512], F32, tag="at", name="at_ps")
                         for _ in range(2)]
                pend = {}

                def emit_scores(g):
                    kts = (2 * g, 2 * g + 1)
                    rss = [max(0, (kt - 4 * qc)) * 128 for kt in kts]
                    stp = [ps_big.tile([128, 1024], F32, tag="big", name="st")
                           for _ in range(2)]
                    for i, kt in enumerate(kts):
                        for h in range(2):
                            hb = slice(h * HD, (h + 1) * HD)
                            nc.tensor.matmul(
                                stp[h][:, i * 512 + rss[i]:(i + 1) * 512],
                                qk_sb[2 + p][hb, kt * 128:(kt + 1) * 128],
                                qk_sb[p][hb, qc * 512 + rss[i]:(qc + 1) * 512],
                                start=True,
                                stop=True,
                            )
                    pp = [p_pool.tile([128, 1024], MM_DT, tag="p", name="pp")
                          for _ in range(2)]
                    for h in range(2):
                        nc.scalar.activation(
                            pp[h][:, rss[0]:1024], stp[h][:, rss[0]:1024], EXP)
                        for i, kt in enumerate(kts):
                            if kt >= 4 * qc:
                                c = i * 512 + rss[i]
                                nc.gpsimd.tensor_mul(
                                    pp[h][:, c:c + 128],
                                    pp[h][:, c:c + 128],
                                    mask_sb,
                                )
                    pend[g] = (pp, kts, rss)

                def emit_attnT(g):
                    pp, kts, rss = pend.pop(g)
                    for h in range(2):
                        for i, kt in enumerate(kts):
                            nc.tensor.matmul(
                                at_ps[h][0:HD + 1, rss[i]:512],
                                v_sb[kt][:, 2 * p + h, :],
                                pp[h][:, i * 512 + rss[i]:(i + 1) * 512],
                                start=(g == 0 and i == 0),
                                stop=(g == G - 1 and i == 1),
                            )

                for g in range(G + 1):
                    if g < G:
                        emit_scores(g)
                    if g >= 1:
                        emit_attnT(g - 1)
                    if fillers:
                        fillers.pop(0)()

                # normalize: evacuate numerator+denominator in one DVE copy,
                # DMA-broadcast the denominator row, reciprocal on DVE, then
                # per-head multiply: head 0 on DVE (base-0 write), head 1 on
                # gpsimd (base-64 write).
                atvs, recs = [], []
                for h in range(2):
                    atv = sm_pool.tile([HD + 1, 512], F32, tag="atv", name="atv")
                    nc.vector.tensor_copy(atv, at_ps[h][0:HD + 1, :])
                    bcr = sm_pool.tile([HD, 512], F32, tag="bcr", name="bcr")
                    nc.sync.dma_start(
                        out=bcr,
                        in_=atv[HD:HD + 1, :].unsqueeze(1).broadcast_to(
                            [1, HD, 512]))
                    rec = sm_pool.tile([HD, 512], F32, tag="rec", name="rec")
                    nc.vector.reciprocal_approx_fast(rec, bcr)
                    atvs.append(atv)
                    recs.append(rec)
                nc.vector.tensor_mul(
                    an[p][0:HD, :], atvs[0][0:HD, :], recs[0])
                nc.gpsimd.tensor_mul(
                    an[p][HD:128, :], atvs[1][0:HD, :], recs[1])

        def outproj_groups(qc):
            an = an_hist[qc]

            def mk(qs):
                def emit():
                    qsl = slice(qs * 128, (qs + 1) * 128)
                    ops = ps_out.tile([128, 1024], F32, tag="out", name="psout")
                    for e in range(2):
                        for p in range(NP):
                            nc.tensor.matmul(
                                ops[:, e * 512:(e + 1) * 512],
                                an[p][:, qsl],
                                owT_sb[p][:, e * 512:(e + 1) * 512],
                                start=(p == 0),
                                stop=(p == NP - 1),
                            )
                    osb = out_pool.tile([128, 1024], F32, tag="osb", name="osb")
                    nc.vector.tensor_copy(osb, ops)
                    nc.sync.dma_start(
                        out=out_d[qc * 512 + qs * 128:qc * 512 + (qs + 1) * 128, :],
                        in_=osb,
                    )
                return emit
            return [mk(qs) for qs in range(4)]

        # ---------------- schedule ----------------
        # qkv s-chunk pair 0 first, then attention chunks [1,2,3,0] with
        # qkv sp1 / previous-chunk outproj groups as PE filler.
        for f in range(4):
            emit_qk_group(f, 0)
        for half in range(4):
            emit_v_group(0, half)

        # v groups first: attention chunk 2 needs v s-tiles 8..11 early.
        fillers = []
        for half in range(4):
            fillers.append(lambda h=half: emit_v_group(1, h))
        for f in range(4):
            fillers.append(lambda f=f: emit_qk_group(f, 1))

        emit_attention(1, fillers)
        fillers += outproj_groups(1)
        emit_attention(2, fillers)
        fillers += outproj_groups(2)
        emit_attention(3, fillers)
        fillers += outproj_groups(3)
        emit_attention(0, fillers)
        fillers += outproj_groups(0)
        for fl in fillers:
            fl()


_CACHE = {}


def _build():
    if "nc" in _CACHE:
        return _CACHE["nc"]
    nc = bacc.Bacc("TRN2", target_bir_lowering=False, debug=False)
    xT_d = nc.dram_tensor("xT", [D, S], MM_DT, kind="ExternalInput").ap()
    wT_d = nc.dram_tensor("wT", [D, 3 * VW], MM_DT, kind="ExternalInput").ap()
    owT_d = nc.dram_tensor("owT", [VW, D], MM_DT, kind="ExternalInput").ap()
    mask_d = nc.dram_tensor("mask", [128, 128], MM_DT, kind="ExternalInput").ap()
    out_d = nc.dram_tensor("out", [S, D], F32, kind="ExternalOutput").ap()
    with tile.TileContext(nc) as tc:
        _emit(tc, nc, xT_d, wT_d, owT_d, mask_d, out_d)
    nc.compile()
    _CACHE["nc"] = nc
    return nc


def _mask_np():
    # [128, 128] 0/1 keep-mask: m[k, q] = 1 if q >= k else 0.
    r = np.arange(128)
    return (r[None, :] >= r[:, None]).astype(np.float32)


def make_in_maps(x, qkv_w, out_w):
    """Per-core input dicts for the 8-way (batch x head-group) sharding."""
    x = np.asarray(x, np.float32)
    qkv_w = np.asarray(qkv_w, np.float32)
    out_w = np.asarray(out_w, np.float32)
    xT = [np.ascontiguousarray(x[b].T) for b in range(B)]
    mask = _mask_np()
    import ml_dtypes
    np_mm = ml_dtypes.bfloat16
    in_maps = []
    for c in range(NCORES):
        b = c // 4
        h0 = (c % 4) * NH
        rows = np.arange(h0 * HD, (h0 + NH) * HD)
        wq = qkv_w[rows] * np.float32(SCALE)
        wk = qkv_w[D + rows]
        wv = qkv_w[2 * D + rows]
        wT = np.ascontiguousarray(np.concatenate([wq, wk, wv], 0).T)
        owT = np.ascontiguousarray(out_w[:, rows].T)
        in_maps.append({"xT": xT[b].astype(np_mm), "wT": wT.astype(np_mm),
                        "owT": owT.astype(np_mm), "mask": mask.astype(np_mm)})
    return in_maps


def kernel(x, qkv_w, out_w, _trace=False, _trace_cores=None):
    nc = _build()
    in_maps = make_in_maps(x, qkv_w, out_w)
    res = run_bass_kernel_spmd(
        nc, in_maps, core_ids=list(range(NCORES)),
        trace=_trace, trace_cores=_trace_cores,
    )
    outs = [r["out"] for r in res.results]
    full = np.stack([
        outs[0] + outs[1] + outs[2] + outs[3],
        outs[4] + outs[5] + outs[6] + outs[7],
    ]).astype(np.float32)
    if _trace:
        return full, res
    return full
